# revision 15
# baseline (speedup 1.0000x reference)
"""Trainium2 Bass kernel for MQA sparse attention (nn_Attention_83356725281353).

Batch-parallel across 8 NeuronCores (4 batches each), no collectives. All
streamed tensors host-cast to bf16. Host staging is pure layout: KV-cache
roll, k transposed to [e, KV], bias pre-permuted AND pre-exponentiated
(exp(bias), so the chip computes a = exp(p) * eb with ACT reading PSUM
directly and DVE doing a cheap bf16 multiply), v permuted per-partition-
contiguous with a ones column appended, 1/sqrt(d) folded into wq.

v3 scheduling (vs the 148-158us baseline):
  - sync HWDGE ring: 8 wq sub-group DMAs first (phase-P pipeline), then
    the kt/vv/eb stream owns it; nothing compute-paced ever sits in front
    of stream DMAs (the baseline head-of-line blocked ~20us there).
  - wo travels on the otherwise-idle gpsimd SWDGE ring; shared weight
    pool lets wo groups reuse wq SBUF after the q-projection read it.
  - rolling-cache update (k_new/v_new) is folded into the attention
    matmuls by splitting the kv contraction [0:124)+[124:128) on the
    last tile - no DVE/DMA writes into streamed tiles (v2's Vector-queue
    reorder stalled 22us on exactly that).
  - a = exp(p)*exp(bias): ACT reads PSUM, DVE multiplies in bf16.
  - phase-P PSUM casts split DVE/ACT and interleaved with per-head
    transposes; output projection uses PSUM-bank-interleaved order with
    split copies and per-block store DMAs.
"""

import numpy as np

B, Q, DIM, H, HD, KV = 32, 4, 2048, 16, 128, 8192
NCORES = 8
BPC = B // NCORES            # 4 batches per core
BQ = BPC * Q                 # 16 (b,q) rows per core
ROWS = H * Q                 # 64 attention rows per batch
NPAIR = BPC // 2             # 2 batch-pairs per core
DT = 16                      # dim tiles (DIM/128)
KCH = 2048                   # kv chunk width
NCH = KV // KCH              # 4 chunks per batch
VW = HD + 1                  # v width incl. ones column
GW = 2 * DIM                 # weight group width (2 tiles of 2048)
BLOB_X = 2 * DT * HD         # xT offset inside the const blob

_CACHE = {}


def _build():
    import concourse.bass as bass
    import concourse.tile as tile
    from concourse import bacc, mybir, masks

    f32 = mybir.dt.float32
    bf16 = mybir.dt.bfloat16

    nc = bacc.Bacc("TRN2", target_bir_lowering=False, debug=False,
                   num_devices=NCORES)

    # blob: [wk_sb | wv_sb | xT_sb] pre-arranged on host
    blob = nc.dram_tensor("blob", [128, BLOB_X + DT * BQ], bf16,
                          kind="ExternalInput").ap()
    # wqh: [p, t, he] = wq[(t p), he]; woh: [p, h, d] = wo[(h p), d]
    wqh = nc.dram_tensor("wqh", [128, DT * DIM], bf16,
                         kind="ExternalInput").ap()
    woh = nc.dram_tensor("woh", [128, H * DIM], bf16,
                         kind="ExternalInput").ap()
    bq = nc.dram_tensor("bq", [1, H * HD], bf16, kind="ExternalInput").ap()
    bk = nc.dram_tensor("bk", [1, HD], bf16, kind="ExternalInput").ap()
    bv = nc.dram_tensor("bv", [1, HD], bf16, kind="ExternalInput").ap()
    bo = nc.dram_tensor("bo", [1, DIM], bf16, kind="ExternalInput").ap()
    kT = nc.dram_tensor("kT", [BPC, HD, KV], bf16, kind="ExternalInput").ap()
    vv = nc.dram_tensor("vv", [BPC, 128, KV // 128, VW], bf16,
                        kind="ExternalInput").ap()
    # eb[j, p, c, (n t r)]: exp(bias), kv = c*2048 + n*512 + t*128 + p
    eb = nc.dram_tensor("eb", [NPAIR, 128, NCH, KCH], bf16,
                        kind="ExternalInput").ap()
    ones = nc.dram_tensor("ones", [1, BQ], bf16, kind="ExternalInput").ap()
    out = nc.dram_tensor("out", [BQ, DIM], f32, kind="ExternalOutput").ap()

    with tile.TileContext(nc) as tc:
        _body(tc, nc, bass, mybir, masks, blob, wqh, woh, bq, bk, bv, bo,
              kT, vv, eb, ones, out)

    nc.compile()
    return nc


def _body(tc, nc, bass, mybir, masks, blob, wqh, woh, bq, bk, bv, bo,
          kT, vv, eb, ones, out):
    from contextlib import ExitStack

    f32 = mybir.dt.float32
    bf16 = mybir.dt.bfloat16
    EXP = mybir.ActivationFunctionType.Exp
    COPY = mybir.ActivationFunctionType.Copy

    with ExitStack() as octx:
        const = octx.enter_context(tc.tile_pool(name="const", bufs=1))
        wpool = octx.enter_context(tc.tile_pool(name="w", bufs=12))
        kpool = octx.enter_context(tc.tile_pool(name="kt", bufs=6))
        vpool = octx.enter_context(tc.tile_pool(name="vt", bufs=6))
        bpool = octx.enter_context(tc.tile_pool(name="eb", bufs=4))
        apool = octx.enter_context(tc.tile_pool(name="a", bufs=6))
        fpool = octx.enter_context(tc.tile_pool(name="fin", bufs=2))

        # wq sub-groups lead BOTH HWDGE rings (a single ring sustains only
        # ~320 GB/s; two together reach ~420), so phase-P finishes ~30us
        wgs = []
        for g in range(8):
            w_t = wpool.tile([128, GW], bf16, tag="wg")
            eng = nc.sync if g % 2 == 0 else nc.scalar
            eng.dma_start(w_t[:], wqh[:, g * GW:(g + 1) * GW])
            wgs.append(w_t)

        ident_f = const.tile([128, 128], f32, tag="idf")
        ident_b = const.tile([128, 128], bf16, tag="idb")
        masks.make_identity(nc, ident_f[:])
        masks.make_identity(nc, ident_b[:])

        # const loads + wo groups on the gpsimd (SWDGE) ring
        blob_sb = const.tile([128, BLOB_X + DT * BQ], bf16, tag="blob")
        nc.gpsimd.dma_start(blob_sb[:], blob)
        ones16 = const.tile([1, BQ], bf16, tag="ones16")
        nc.gpsimd.dma_start(ones16[:], ones)
        vn_sb = const.tile([BQ, HD], bf16, tag="vn")
        bq_sb = const.tile([1, H * HD], bf16, tag="bq")
        nc.gpsimd.dma_start(bq_sb[:], bq)
        bk_sb = const.tile([1, HD], bf16, tag="bk")
        nc.gpsimd.dma_start(bk_sb[:], bk)
        bv_sb = const.tile([1, HD], bf16, tag="bv")
        nc.gpsimd.dma_start(bv_sb[:], bv)
        bo_sb = const.tile([1, DIM], bf16, tag="bo")
        nc.gpsimd.dma_start(bo_sb[:], bo)

        q_sb = const.tile([BQ, H * HD], bf16, tag="q")
        kn_sb = const.tile([BQ, HD], bf16, tag="kn")
        # qT layout: [e, (b, h, q)] col = b*64 + h*4 + q
        qT_sb = const.tile([128, BPC * ROWS], bf16, tag="qT")
        knT_sb = const.tile([128, BQ], bf16, tag="knT")
        # oT layout: [e=128, (h,b,q)] col = h*16 + b*4 + q
        oT_sb = const.tile([128, BPC * ROWS], bf16, tag="oT")

        # ---------------- Phase P: projections -----------------------------
        with (tc.tile_pool(name="qps", bufs=1, space="PSUM") as qps,
              tc.tile_pool(name="ptr", bufs=1, space="PSUM") as ptr):
            q_ps = qps.tile([BQ, H * HD], f32, tag="qacc")
            kv_ps = qps.tile([BQ, 2 * HD], f32, tag="kvacc")
            for t in range(DT):
                w_t = wgs[t // 2]
                wof = (t % 2) * DIM
                lhs = blob_sb[:, BLOB_X + t * BQ:BLOB_X + (t + 1) * BQ]
                for n in range(4):
                    nc.tensor.matmul(q_ps[:, n * 512:(n + 1) * 512], lhs,
                                     w_t[:, wof + n * 512:wof + (n + 1) * 512],
                                     start=(t == 0), stop=False)
                nc.tensor.matmul(kv_ps[:, 0:HD], lhs,
                                 blob_sb[:, t * HD:(t + 1) * HD],
                                 start=(t == 0), stop=False)
                nc.tensor.matmul(kv_ps[:, HD:2 * HD], lhs,
                                 blob_sb[:, DT * HD + t * HD:
                                         DT * HD + (t + 1) * HD],
                                 start=(t == 0), stop=False)
            # bias rows via ones-row matmul (K=1); biases are zeros for this
            # problem but kept for fidelity
            ones_r = ones16[0:1, :]
            for n in range(4):
                nc.tensor.matmul(q_ps[:, n * 512:(n + 1) * 512], ones_r,
                                 bq_sb[0:1, n * 512:(n + 1) * 512],
                                 start=False, stop=True)
            nc.tensor.matmul(kv_ps[:, 0:HD], ones_r, bk_sb[0:1, :],
                             start=False, stop=True)
            nc.tensor.matmul(kv_ps[:, HD:2 * HD], ones_r,
                             bv_sb[0:1, :], start=False, stop=True)

            # PSUM -> SBUF casts split DVE/ACT, interleaved with per-head
            # transposes so qT is ready ~3us after the last q matmul
            qtr = ptr.tile([128, H * BQ], bf16, tag="qtr")
            for n in range(4):
                dst = q_sb[:, n * 512:(n + 1) * 512]
                src = q_ps[:, n * 512:(n + 1) * 512]
                if n % 2 == 0:
                    nc.vector.tensor_copy(dst, src)
                else:
                    nc.scalar.activation(dst, src, COPY)
                for h in range(4 * n, 4 * n + 4):
                    nc.tensor.transpose(qtr[:, h * BQ:(h + 1) * BQ],
                                        q_sb[:, h * HD:(h + 1) * HD],
                                        ident_b[0:BQ, 0:BQ])
            nc.vector.tensor_copy(kn_sb[:], kv_ps[:, 0:HD])
            nc.vector.tensor_copy(vn_sb[:], kv_ps[:, HD:2 * HD])

            qtr_hbq = qtr[:].rearrange("p (h b q) -> p h b q", h=H, b=BPC)
            for b in range(BPC):
                dst = qT_sb[:, b * ROWS:(b + 1) * ROWS].rearrange(
                    "p (h q) -> p h q", h=H)
                nc.vector.tensor_copy(dst, qtr_hbq[:, :, b, :])
            trk = ptr.tile([128, BQ], bf16, tag="tr")
            nc.tensor.transpose(trk[:], kn_sb[:], ident_b[0:BQ, 0:BQ])
            nc.vector.tensor_copy(knT_sb[:], trk[:])

        # ---------------- Phase A: attention, per batch-pair ---------------
        # p^T layout: kT tiles are the stationary, so the prob tiles feed
        # the o-matmul directly. Softmax denominators come from the ones
        # column appended to v on the host (o_ps col 128). The rolling
        # cache's 4 new tokens ride in via contraction splits on the last
        # kv tile (kv 8188..8191 = partitions 124..128 of the last block).
        # v tiles ride the scalar ring prefetched one chunk ahead (so the
        # ACT queue's exp instructions never head-of-line block them);
        # kt/eb ride the sync ring; wo groups fill ring slack mid-stream
        wos = [None] * 8
        vt = {}

        def _issue_v(it):
            jj, cc = it // NCH, it % NCH
            bb0, bb1 = 2 * jj, 2 * jj + 1
            v0 = vpool.tile([128, 16 * VW], bf16, tag="vt")
            nc.scalar.dma_start(
                v0[:].rearrange("p (n e) -> p n e", n=16),
                vv[bb0][:, cc * 16:(cc + 1) * 16, :])
            v1 = vpool.tile([128, 16 * VW], bf16, tag="vt")
            nc.scalar.dma_start(
                v1[:].rearrange("p (n e) -> p n e", n=16),
                vv[bb1][:, cc * 16:(cc + 1) * 16, :])
            vt[it] = (v0, v1)

        def _issue_wo(g, eng):
            w_t = wpool.tile([128, GW], bf16, tag="wg")
            eng.dma_start(w_t[:], woh[:, g * GW:(g + 1) * GW])
            wos[g] = w_t

        _issue_v(0)
        with (tc.tile_pool(name="pps", bufs=4, space="PSUM") as pps,
              tc.tile_pool(name="tps", bufs=2, space="PSUM") as tps,
              tc.tile_pool(name="ops", bufs=2, space="PSUM") as ops):
            for j in range(NPAIR):
                b0, b1 = 2 * j, 2 * j + 1
                o_ps = ops.tile([128, VW], f32, tag="o")
                for c in range(NCH):
                    it = j * NCH + c
                    kt0 = kpool.tile([128, KCH], bf16, tag="kt")
                    nc.sync.dma_start(kt0[:], kT[b0][:, c * KCH:(c + 1) * KCH])
                    kt1 = kpool.tile([128, KCH], bf16, tag="kt")
                    nc.sync.dma_start(kt1[:], kT[b1][:, c * KCH:(c + 1) * KCH])
                    eb_sb = bpool.tile([128, KCH], bf16, tag="eb")
                    nc.sync.dma_start(eb_sb[:], eb[j][:, c, :])
                    if it + 1 < NPAIR * NCH:
                        _issue_v(it + 1)
                    if 1 <= it <= 6:
                        _issue_wo(it - 1, nc.scalar)
                    if 5 <= it <= 6:
                        _issue_wo(it + 1, nc.sync)
                    v0, v1 = vt[it]
                    for n in range(4):
                        if c == NCH - 1 and n == 3:
                            # rolling-cache injection of the new k/v tokens,
                            # emitted HERE (not at chunk top) so the Vector/
                            # GpSimd queues never block earlier sub-chunks
                            nc.vector.tensor_copy(
                                kt0[:, KCH - 4:KCH],
                                knT_sb[:, b0 * 4:b0 * 4 + 4])
                            nc.vector.tensor_copy(
                                kt1[:, KCH - 4:KCH],
                                knT_sb[:, b1 * 4:b1 * 4 + 4])
                            nc.gpsimd.dma_start(
                                v0[124:128, 15 * VW:15 * VW + HD],
                                vn_sb[b0 * 4:b0 * 4 + 4, :])
                            nc.gpsimd.dma_start(
                                v1[124:128, 15 * VW:15 * VW + HD],
                                vn_sb[b1 * 4:b1 * 4 + 4, :])
                        p_ps = pps.tile([128, 512], f32, tag="p")
                        for t in range(4):
                            ko = (n * 4 + t) * 128
                            nc.tensor.matmul(
                                p_ps[:, t * 128:t * 128 + ROWS],
                                kt0[:, ko:ko + 128],
                                qT_sb[:, b0 * ROWS:(b0 + 1) * ROWS],
                                start=True, stop=True)
                            nc.tensor.matmul(
                                p_ps[:, t * 128 + ROWS:(t + 1) * 128],
                                kt1[:, ko:ko + 128],
                                qT_sb[:, b1 * ROWS:(b1 + 1) * ROWS],
                                start=True, stop=True)
                        # a = exp(p) * exp(bias): ACT reads PSUM, DVE does
                        # a bf16 multiply (2x DVE rate vs the f32 add)
                        e_bf = apool.tile([128, 512], bf16, tag="e")
                        nc.scalar.activation(e_bf[:], p_ps[:], EXP)
                        a_bf = apool.tile([128, 512], bf16, tag="abf")
                        nc.vector.tensor_tensor(
                            a_bf[:], e_bf[:],
                            eb_sb[:, n * 512:(n + 1) * 512],
                            op=mybir.AluOpType.mult)
                        for t in range(4):
                            kvt = c * 16 + n * 4 + t
                            first, last = (kvt == 0), (kvt == 63)
                            vo = (n * 4 + t) * VW
                            nc.tensor.matmul(
                                o_ps[0:ROWS, :],
                                a_bf[:, t * 128:t * 128 + ROWS],
                                v0[:, vo:vo + VW], start=first, stop=last)
                            nc.tensor.matmul(
                                o_ps[ROWS:128, :],
                                a_bf[:, t * 128 + ROWS:(t + 1) * 128],
                                v1[:, vo:vo + VW], start=first, stop=last,
                                tile_position=(0, 64))
                _finalize_pair(tc, nc, mybir, fpool, tps, j, o_ps, oT_sb,
                               ident_f)
                if j == NPAIR - 1:
                    # HAM keep-warm: PE activity through the finalize drain
                    # so the output projection runs at full clock
                    for _ in range(4):
                        d_ps = pps.tile([128, 512], f32, tag="p")
                        nc.tensor.matmul(d_ps[:, :], ident_b[:],
                                         eb_sb[:, 0:512],
                                         start=True, stop=True)

        # ---------------- Phase O: output projection ------------------------
        # h-outer keeps consecutive matmuls on different PSUM banks (drain
        # overlap); copies split DVE/ACT, stores per 512-block
        ones_r = ones16[0:1, :]
        with tc.tile_pool(name="outps", bufs=1, space="PSUM") as outps:
            out_ps = outps.tile([BQ, DIM], f32, tag="out")
            for h in range(H):
                w_t = wos[h // 2]
                wof = (h % 2) * DIM
                lhs = oT_sb[:, h * BQ:(h + 1) * BQ]
                for n in range(4):
                    nc.tensor.matmul(out_ps[:, n * 512:(n + 1) * 512], lhs,
                                     w_t[:, wof + n * 512:wof + (n + 1) * 512],
                                     start=(h == 0), stop=False)
            for n in range(4):
                nc.tensor.matmul(out_ps[:, n * 512:(n + 1) * 512], ones_r,
                                 bo_sb[0:1, n * 512:(n + 1) * 512],
                                 start=False, stop=True)
                dst = fpool.tile([BQ, 512], f32, tag="outnb")
                if n % 2 == 0:
                    nc.vector.tensor_copy(dst[:], out_ps[:, n * 512:
                                                         (n + 1) * 512])
                else:
                    nc.scalar.activation(dst[:],
                                         out_ps[:, n * 512:(n + 1) * 512],
                                         COPY)
                nc.sync.dma_start(out[:, n * 512:(n + 1) * 512], dst[:])


def _finalize_pair(tc, nc, mybir, fpool, tps, j, o_ps, oT_sb, ident_f):
    f32 = mybir.dt.float32
    recip = fpool.tile([128, 1], f32, tag="recip")
    nc.vector.reciprocal(recip[:], o_ps[:, HD:HD + 1])
    o_sb = fpool.tile([128, HD], f32, tag="osb")
    nc.vector.tensor_scalar_mul(o_sb[:], o_ps[:, 0:HD], recip[:])
    tr = tps.tile([128, 128], f32, tag="tr")
    nc.tensor.transpose(tr[:], o_sb[:], ident_f[:])
    oT_4d = oT_sb[:].rearrange("p (h b q) -> p h b q", h=H, b=BPC)
    for b2 in range(2):
        nc.vector.tensor_copy(
            oT_4d[:, :, 2 * j + b2, :],
            tr[:, b2 * ROWS:(b2 + 1) * ROWS].rearrange(
                "p (h q) -> p h q", h=H))


def _get_nc():
    if "nc" not in _CACHE:
        _CACHE["nc"] = _build()
    return _CACHE["nc"]


def kernel(x, attn_bias, cache_k, cache_v, wq, bq, wk, bk, wv, bv, wo, bo):
    import ml_dtypes
    from concourse.bass_utils import run_bass_kernel_spmd

    nc = _get_nc()
    scale = np.float32(1.0 / np.sqrt(HD))
    bf = ml_dtypes.bfloat16

    x = np.asarray(x, np.float32)
    xT_full = np.ascontiguousarray(x.reshape(B * Q, DIM).T).astype(bf)
    # wq with 1/sqrt(d) folded, [p, t, he]
    wqh = np.ascontiguousarray(
        (np.asarray(wq, np.float32) * scale).reshape(DT, 128, H * HD)
        .transpose(1, 0, 2).reshape(128, DT * H * HD)).astype(bf)
    bq2 = np.ascontiguousarray(
        (np.asarray(bq, np.float32) * scale).reshape(1, H * HD)).astype(bf)
    wk2 = np.asarray(wk, np.float32).reshape(DT, 128, HD)
    wv2 = np.asarray(wv, np.float32).reshape(DT, 128, HD)
    bk2 = np.asarray(bk, np.float32).reshape(1, HD).astype(bf)
    bv2 = np.asarray(bv, np.float32).reshape(1, HD).astype(bf)
    kTh = np.ascontiguousarray(
        np.roll(np.asarray(cache_k, np.float32), -Q, axis=1)
        .transpose(0, 2, 1)).astype(bf)
    vr0 = np.roll(np.asarray(cache_v, np.float32), -Q, axis=1)
    # [B, KV, HD] -> [B, 128, KV/128, HD+1]: per-partition-contiguous runs,
    # last column = 1.0 so the o-matmul accumulates softmax denominators
    vrh4 = vr0.reshape(B, KV // 128, 128, HD).transpose(0, 2, 1, 3)
    vrh = np.ones((B, 128, KV // 128, VW), np.float32)
    vrh[..., :HD] = vrh4
    vrh = np.ascontiguousarray(vrh).astype(bf)
    # exp(bias) -> [pair, p, c, (n t r)] with kv = c*2048 + n*512 + t*128 + p
    ab = np.exp(np.asarray(attn_bias, np.float32)).reshape(
        B // 2, 2, ROWS, KV)
    abP = ab.transpose(0, 3, 1, 2).reshape(B // 2, KV, 2 * ROWS)
    ebP = np.ascontiguousarray(
        abP.reshape(B // 2, NCH, 4, 4, 128, 2 * ROWS)
        .transpose(0, 4, 1, 2, 3, 5)
        .reshape(B // 2, 128, NCH, KCH)).astype(bf)
    # wo [p, h, d]
    woh = np.ascontiguousarray(
        np.asarray(wo, np.float32).reshape(H * HD, DIM)
        .reshape(H, 128, DIM).transpose(1, 0, 2)
        .reshape(128, H * DIM)).astype(bf)
    bo2 = np.asarray(bo, np.float32).reshape(1, DIM).astype(bf)

    # const blob: [wk_sb | wv_sb | xT_sb-slot]; xT slot filled per core
    wk_sb = wk2.transpose(1, 0, 2).reshape(128, DT * HD)
    wv_sb = wv2.transpose(1, 0, 2).reshape(128, DT * HD)

    in_maps = []
    for c in range(NCORES):
        xc = xT_full[:, c * BQ:(c + 1) * BQ].astype(np.float32)
        xT_sb = xc.reshape(DT, 128, BQ).transpose(1, 0, 2).reshape(
            128, DT * BQ)
        blob = np.empty((128, BLOB_X + DT * BQ), np.float32)
        blob[:, 0:DT * HD] = wk_sb
        blob[:, DT * HD:BLOB_X] = wv_sb
        blob[:, BLOB_X:] = xT_sb
        in_maps.append({
            "blob": np.ascontiguousarray(blob).astype(bf),
            "wqh": wqh, "woh": woh,
            "bq": bq2, "bk": bk2, "bv": bv2, "bo": bo2,
            "kT": np.ascontiguousarray(kTh[c * BPC:(c + 1) * BPC]),
            "vv": np.ascontiguousarray(vrh[c * BPC:(c + 1) * BPC]),
            "eb": np.ascontiguousarray(ebP[NPAIR * c:NPAIR * (c + 1)]),
            "ones": np.ones((1, BQ), bf),
        })

    res = run_bass_kernel_spmd(nc, in_maps, core_ids=list(range(NCORES)))
    _CACHE["last_result"] = res
    outs = [res.results[c]["out"] for c in range(NCORES)]
    return np.concatenate(outs, axis=0).reshape(B, Q, DIM).astype(np.float32)
